# revision 23
# baseline (speedup 1.0000x reference)
"""Causal self-attention (B=2, T=2048, C=1024, NH=16, HD=64) on 8 TRN2 cores.

Sharding: core c -> batch b = c//4, head group j = c%4 (4 heads: 4j..4j+3).
Each core computes its batch's QKV projection for its 4 heads, rope, causal
attention in S^T layout (k on partitions, q on free dim), and a partial
output projection y_part^T = Wp_slice^T.T @ attT. Host sums the 4 per-batch
partials and adds b_proj.

Fused per-qc pipeline: for each 512-token q chunk, attention (both head
pairs), rowsum-normalize, output projection and DMA-out all overlap with the
next chunk's attention. QK/V projection blocks are interleaved as PE filler
work into the exp-bound gaps of the attention loop.

Device layouts (per core, t = 2048 tokens of its batch):
  xT   [128, 8, 2048]  bf16   x[b].T tiled over 8 c-tiles
  q/k  [128, 2, 2048]  bf16   head-pair dims on partitions, rope applied
  v    [128, 64, 65]   bf16   [tok-part, u=(tt,hp,h), 64 dims + ones col]
  S^T  psum [128, 1024] f32   [kt 128 x (h0 512q | h1 512q)]
  P^T  [128, 1024] bf16       exp(S^T/8), causal-masked on DVE
  PV   psum [65, 512] x2      rows 0-63 att^T, row 64 rowsum (ones col)
  attbf [128, 2, 512] bf16    per-qc normalized att^T, consumed by E
  yT   [1024, 2048] bf16      per-qc column blocks DMA'd as produced
"""
import numpy as np
import ml_dtypes
from contextlib import ExitStack

import concourse.bass as bass
import concourse.mybir as mybir
import concourse.tile as tile
from concourse import bacc
from concourse.bass_utils import run_bass_kernel_spmd

F32 = mybir.dt.float32
BF16 = mybir.dt.bfloat16
AF = mybir.ActivationFunctionType
ALU = mybir.AluOpType

B, T, C = 2, 2048, 1024
NH, HD = 16, 64
TL = 2048          # per-core token count (one batch)
NCT = C // 128     # 8 contraction tiles
NTC = TL // 512    # 4 q-chunks of 512
NTT = TL // 128    # 16 token tiles of 128

TRACE = False      # set by test harness for profiling runs
_CACHE = {}


def _build_nc():
    nc = bacc.Bacc("TRN2", target_bir_lowering=False, debug=False)
    xT_d = nc.dram_tensor("xT", [128, NCT, TL], BF16, kind="ExternalInput").ap()
    wqk_d = nc.dram_tensor("wqkT", [128, 4, NCT, 128], BF16, kind="ExternalInput").ap()
    wv_d = nc.dram_tensor("wvT", [128, NCT, 256], BF16, kind="ExternalInput").ap()
    bqk_d = nc.dram_tensor("bqk", [128, 4], F32, kind="ExternalInput").ap()
    bv_d = nc.dram_tensor("bv", [128, 256], F32, kind="ExternalInput").ap()
    rope_d = nc.dram_tensor("rope", [128, TL], BF16, kind="ExternalInput").ap()
    masks_d = nc.dram_tensor("masks", [128, 4, 1024], BF16, kind="ExternalInput").ap()
    wp_d = nc.dram_tensor("wpT", [128, 2, 1024], BF16, kind="ExternalInput").ap()
    yT_d = nc.dram_tensor("yT", [1024, TL], BF16, kind="ExternalOutput").ap()
    rs_dram = nc.dram_tensor("rs_scratch", [4, TL], F32)

    with tile.TileContext(nc) as tc, ExitStack() as ctx:
        sb = ctx.enter_context(tc.tile_pool(name="sb", bufs=1))
        ptp = ctx.enter_context(tc.tile_pool(name="ptp", bufs=6))
        abp = ctx.enter_context(tc.tile_pool(name="abp", bufs=2))
        atp = ctx.enter_context(tc.tile_pool(name="atp", bufs=4))
        rbp = ctx.enter_context(tc.tile_pool(name="rbp", bufs=4))
        ytp = ctx.enter_context(tc.tile_pool(name="ytp", bufs=4))

        xT = sb.tile([128, NCT, TL], BF16)
        wqk = sb.tile([128, 4, NCT, 128], BF16)
        wv = sb.tile([128, NCT, 256], BF16)
        bqk = sb.tile([128, 4], F32)
        bv = sb.tile([128, 256], F32)
        rope = sb.tile([128, TL], BF16)
        masks = sb.tile([128, 4, 1024], BF16)
        wp = sb.tile([128, 2, 1024], BF16)
        q_sb = sb.tile([128, 2, TL], BF16)
        k_sb = sb.tile([128, 2, TL], BF16)
        v_sb = sb.tile([128, 4 * NTT, 65], BF16)
        rs_sb = sb.tile([128, 512], F32)    # rows 0/32 <- rowsums of h0/h1
        rsr_sb = sb.tile([128, 512], F32)   # reciprocal of rs_sb
        ones_sb = sb.tile([128, 64], BF16)  # lhsT for PE rowsum broadcast
        rsrb_sb = sb.tile([128, 512], BF16)  # bf16 recip rows (tail bcast rhs)
        rsc_sb = sb.tile([128, 512], F32)   # PE-broadcast recip (last chunk)

        # ---- input DMA, priority-ordered ----
        # DMA_DIRECT2D occupies the issuing ring for the transfer. Wave 1
        # (everything the first blocks + D(0,hp0) start need) fans out over
        # 5 rings; tensor/vector only carry wave 1 so their compute streams
        # aren't delayed. Waves 2/3 round-robin sync/gpsimd/scalar.
        nc.vector.memset(v_sb[:, :, 64:65], 1.0)
        nc.vector.memset(rs_sb, 1.0)  # keep unused partitions finite for recip
        nc.vector.memset(ones_sb, 1.0)

        r1 = [nc.sync, nc.gpsimd, nc.scalar]
        ri = [0]

        def dma1(out, in_):
            r1[ri[0] % 3].dma_start(out=out, in_=in_)
            ri[0] += 1

        dma1(bqk, bqk_d)
        dma1(wqk[:, 2], wqk_d[:, 2])
        dma1(wqk[:, 0], wqk_d[:, 0])
        for half in range(2):
            dma1(wv[:, half * 4:(half + 1) * 4, :],
                 wv_d[:, half * 4:(half + 1) * 4, :])
        for ct in range(NCT):  # x for q-chunk 0
            dma1(xT[:, ct, 0:512], xT_d[:, ct, 0:512])
        dma1(bv, bv_d)
        dma1(rope, rope_d)
        dma1(masks[:, 0, :], masks_d[:, 0, :])

        r2 = [nc.sync, nc.gpsimd, nc.scalar]
        ri2 = [0]

        def dma(out, in_):
            r2[ri2[0] % 3].dma_start(out=out, in_=in_)
            ri2[0] += 1

        for m in (3, 1):       # head pair 1 weights
            dma(wqk[:, m], wqk_d[:, m])
        for d in range(1, 4):
            dma(masks[:, d, :], masks_d[:, d, :])
        for tc4 in (1, 2, 3):
            for ct in range(NCT):
                sl = slice(tc4 * 512, (tc4 + 1) * 512)
                dma(xT[:, ct, sl], xT_d[:, ct, sl])
            if tc4 == 1:
                dma(wp[:, 0, :], wp_d[:, 0, :])
                dma(wp[:, 1, :], wp_d[:, 1, :])

        with tc.tile_pool(name="pds", bufs=2, space="PSUM") as pds, \
             tc.tile_pool(name="pdv", bufs=1, space="PSUM") as pdv, \
             tc.tile_pool(name="fill", bufs=1, space="PSUM") as fill:

            def c_tile(tt):
                """V projection for one token tile (8 MMs + fused bias add)."""
                ps = fill.tile([128, 512], F32, tag=f"f{tt % 2}",
                               name=f"pc_{tt}")[:, 0:256]
                for ct in range(NCT):
                    nc.tensor.matmul(
                        ps, xT[:, ct, tt * 128:(tt + 1) * 128], wv[:, ct, :],
                        start=(ct == 0), stop=(ct == NCT - 1),
                        skip_group_check=True)
                nc.vector.tensor_add(
                    v_sb[:, 4 * tt:4 * tt + 4, 0:64],
                    ps.rearrange("p (a b) -> p a b", a=4),
                    bv.rearrange("p (a b) -> p a b", a=4))

            def b_block(m, tc4):
                """QK projection for one 512-col q/k chunk of m-tile m."""
                sl = slice(tc4 * 512, (tc4 + 1) * 512)
                ps = fill.tile([128, 512], F32, tag=f"f{m % 2}",
                               name=f"pb_{m}_{tc4}")
                for ct in range(NCT):
                    nc.tensor.matmul(
                        ps, wqk[:, m, ct, :], xT[:, ct, sl],
                        start=(ct == 0), stop=(ct == NCT - 1),
                        skip_group_check=True)
                dest = q_sb if m < 2 else k_sb
                nc.vector.scalar_tensor_tensor(
                    out=dest[:, m % 2, sl], in0=ps, scalar=bqk[:, m:m + 1],
                    in1=rope[:, sl], op0=ALU.add, op1=ALU.mult)

            def norm(qc, hp, pv):
                """Copy att^T out of psum (releases pv fast; split DVE/ACT so
                the next head pair's PV never stalls), rowsum recip + recip
                broadcast; returns (attT, rsb) for the final mul.
                Cross-partition copies keep 32-aligned partition shifts.
                Broadcast goes through a DRAM bounce (latency hidden by the
                pipeline) except for the very last chunk, where a PE
                broadcast matmul keeps the chain short and the PE warm."""
                qsl = slice(qc * 512, (qc + 1) * 512)
                att = atp.tile([128, 512], F32, tag="att", name=f"att_{qc}_{hp}")
                nc.vector.tensor_copy(att[0:64, :], pv[0][0:64, :])
                nc.vector.tensor_copy(rs_sb[0:1, :], pv[0][64:65, :])
                nc.scalar.copy(att[64:128, :], pv[1][0:64, :])
                nc.scalar.copy(rs_sb[32:33, :], pv[1][64:65, :])
                nc.vector.reciprocal_approx_fast(rsr_sb, rs_sb)
                if qc == 3 and hp == 1:
                    nc.scalar.copy(rsrb_sb[0:33, :], rsr_sb[0:33, :])
                    bc = fill.tile([128, 512], F32, tag="f0", name="bc_tail")
                    for h in range(2):
                        nc.tensor.matmul(
                            bc[h * 64:(h + 1) * 64, :],
                            ones_sb[32 * h:32 * h + 1, :],
                            rsrb_sb[32 * h:32 * h + 1, :])
                    nc.vector.tensor_copy(rsc_sb, bc)
                    return att, rsc_sb
                rsb = rbp.tile([128, 512], F32, tag="rsb", name=f"rsb_{qc}_{hp}")
                for h in range(2):
                    u4 = 2 * hp + h
                    nc.gpsimd.dma_start(
                        out=rs_dram[u4:u4 + 1, qsl], in_=rsr_sb[32 * h:32 * h + 1, :])
                    bc_ap = bass.AP(tensor=rs_dram, offset=u4 * TL + qc * 512,
                                    ap=[[0, 64], [1, 512]])
                    nc.gpsimd.dma_start(out=rsb[h * 64:(h + 1) * 64, :], in_=bc_ap)
                return att, rsb

            def attbf_mul(ab, hp, att, rsb):
                nc.vector.tensor_mul(ab[:, hp, :], att, rsb)

            def e_mt(qc, mt, ab):
                """Output projection for one 128-row m-tile of this q chunk."""
                ps = fill.tile([128, 512], F32, tag=f"f{mt % 2}",
                               name=f"pe_{qc}_{mt}")
                for hp in range(2):
                    nc.tensor.matmul(
                        ps, wp[:, hp, mt * 128:(mt + 1) * 128], ab[:, hp, :],
                        start=(hp == 0), stop=(hp == 1), skip_group_check=True)
                yt = ytp.tile([128, 512], BF16, tag="yt", name=f"yt_{qc}_{mt}")
                nc.vector.tensor_copy(yt, ps)
                nc.sync.dma_start(
                    out=yT_d[mt * 128:(mt + 1) * 128, qc * 512:(qc + 1) * 512],
                    in_=yt)

            # ---- PE warm-up: dependency-free dummy matmuls bridge the
            # input-DMA wait so HAM un-throttles before real work starts
            # (burst ends well before the first real MM's deps land) ----
            dmy = fill.tile([128, 512], F32, tag="f0", name="warmup")
            for _ in range(64):
                nc.tensor.matmul(dmy[0:64, 0:64], ones_sb[0:1, 0:64],
                                 ones_sb[0:1, 0:64], skip_group_check=True)

            # ---- prework: only what D(0,hp0,kt0) needs; the rest of
            # chunk 0's blocks drain as early fillers (2 pops/group) ----
            b_block(2, 0)
            b_block(0, 0)
            c_tile(0)
            fillers = [
                lambda: c_tile(1), lambda: b_block(3, 0),
                lambda: c_tile(2), lambda: b_block(1, 0),
                lambda: c_tile(3),
            ]

            def pop_filler(n=1):
                for _ in range(n):
                    if fillers:
                        fillers.pop(0)()

            ab = None
            for qc in range(4):
                qsl = slice(qc * 512, (qc + 1) * 512)
                n_kt = 4 * (qc + 1)
                # QK/V blocks for the NEXT q chunk drain as fillers during
                # this chunk's kt groups, so they're emitted (and done)
                # before D(qc+1) reads them. They go to the queue FRONT so
                # deferred normalize muls / E chunks of qc-1 (whose rsb DMA
                # needs time to land) pop later.
                if qc < 3:
                    bc_next = (
                        [(lambda m=m, t=qc + 1: b_block(m, t))
                         for m in (0, 2, 1, 3)]
                        + [(lambda tt=tt: c_tile(tt))
                           for tt in range(4 * qc + 4, 4 * qc + 8)])
                    # qc 0: chunk-0 prework fillers must stay first
                    fillers = (fillers + bc_next if qc == 0
                               else bc_next + fillers)
                for hp in range(2):
                    pv = [pdv.tile([65, 512], F32, tag=f"pv{h}",
                                   name=f"pv_{qc}_{hp}_{h}")
                          for h in range(2)]
                    for kt in range(n_kt):
                        ksl = slice(kt * 128, (kt + 1) * 128)
                        # diagonal staircase: tile kt covers k in
                        # [128kt,128kt+128); q cols j < 128d of this chunk
                        # get zero contribution, so S/exp/mask/PV all shrink
                        # to the valid cols [128d:512).
                        d = kt - 4 * qc  # >= 0 on the diagonal band
                        off = 128 * d if d > 0 else 0
                        sps = pds.tile([128, 1024], F32, tag="sps",
                                       name=f"sps_{qc}_{hp}_{kt}")
                        for h in range(2):
                            hsl = slice(h * 64, (h + 1) * 64)
                            nc.tensor.matmul(
                                sps[:, h * 512 + off:(h + 1) * 512],
                                k_sb[hsl, hp, ksl],
                                q_sb[hsl, hp, qc * 512 + off:(qc + 1) * 512])
                        pt = ptp.tile([128, 1024], BF16, tag="pt",
                                      name=f"pt_{qc}_{hp}_{kt}")
                        if d >= 2:  # split exp pays once >=256 cols are dead
                            for h in range(2):
                                hs = slice(h * 512 + off, (h + 1) * 512)
                                nc.scalar.activation(
                                    pt[:, hs], sps[:, hs], AF.Exp,
                                    bias=0.0, scale=0.125)
                        else:
                            nc.scalar.activation(pt, sps, AF.Exp,
                                                 bias=0.0, scale=0.125)
                        if d == 0:
                            nc.vector.tensor_mul(pt, pt, masks[:, 0, :])
                        elif d > 0:
                            for h in range(2):
                                hs = slice(h * 512 + off, (h + 1) * 512)
                                nc.vector.tensor_mul(
                                    pt[:, hs], pt[:, hs], masks[:, d, hs])
                        for h in range(2):
                            u = (kt * 2 + hp) * 2 + h
                            nc.tensor.matmul(
                                pv[h][:, off:512],
                                v_sb[:, u, :],
                                pt[:, h * 512 + off:(h + 1) * 512],
                                start=(kt == 0), stop=(kt == n_kt - 1),
                                skip_group_check=True)
                        pop_filler(2 if qc == 0 else 1)
                    att, rsb = norm(qc, hp, pv)
                    if hp == 0:
                        ab = abp.tile([128, 2, 512], BF16, tag="ab",
                                      name=f"ab_{qc}")
                    # deferred: pops after next chunk's B/C blocks, by which
                    # time the rsb broadcast DMA has landed (no DVE stall)
                    fillers.append(
                        lambda ab=ab, hp=hp, att=att, rsb=rsb:
                        attbf_mul(ab, hp, att, rsb))
                # projection of this q chunk interleaves into the next chunk
                fillers += [(lambda qc=qc, mt=mt, ab=ab: e_mt(qc, mt, ab))
                            for mt in range(8)]
            for f in fillers:
                f()
    nc.compile()
    return nc


def _rope_T():
    theta = 1.0 / (10000.0 ** (2.0 * np.arange(0, HD // 2, dtype=np.float32) / HD))
    seq = np.arange(1, T + 1, dtype=np.float32)
    ang = np.einsum('n,d->nd', seq, theta)
    ang = np.concatenate([ang, ang], axis=-1)
    f = (np.cos(ang) + np.sin(ang)).astype(np.float32)  # [T, 64]
    return np.concatenate([f.T, f.T], axis=0)           # [128, T]


def _host_inputs(x, W_attn, b_attn, W_proj, b_proj):
    bf = ml_dtypes.bfloat16
    ropeT = _rope_T().astype(bf)
    masks = np.empty((128, 4, 1024), dtype=bf)
    kp = np.arange(128)[:, None]
    qf = np.arange(512)[None, :]
    for d in range(4):
        m = ((kp + 128 * d) <= qf).astype(np.float32)
        masks[:, d, :] = np.concatenate([m, m], axis=1).astype(bf)

    in_maps = []
    for c in range(8):
        b, j = divmod(c, 4)
        hs = [4 * j + i for i in range(4)]
        xT = np.ascontiguousarray(x[b].T).astype(bf)          # [1024, TL]
        q_rows = np.concatenate([W_attn[64 * h:64 * (h + 1)] for h in hs], 0)
        k_rows = np.concatenate([W_attn[C + 64 * h:C + 64 * (h + 1)] for h in hs], 0)
        WqkT = np.concatenate([q_rows, k_rows], 0).T          # [1024, 512]
        bqk = np.concatenate(
            [np.concatenate([b_attn[64 * h:64 * (h + 1)] for h in hs]),
             np.concatenate([b_attn[C + 64 * h:C + 64 * (h + 1)] for h in hs])])
        v_rows = np.concatenate([W_attn[2 * C + 64 * h:2 * C + 64 * (h + 1)] for h in hs], 0)
        WvT = v_rows.T                                        # [1024, 256]
        bv = np.concatenate([b_attn[2 * C + 64 * h:2 * C + 64 * (h + 1)] for h in hs])
        WpT = np.concatenate([W_proj[:, 64 * h:64 * (h + 1)] for h in hs], 1).T  # [256,1024]
        in_maps.append({
            "xT": np.ascontiguousarray(
                xT.reshape(NCT, 128, TL).transpose(1, 0, 2)),
            # [128, 4, NCT, 128]: wqkT[p, m, ct, j] = WqkT[ct*128+p, m*128+j]
            "wqkT": np.ascontiguousarray(
                WqkT.astype(bf).reshape(NCT, 128, 4, 128).transpose(1, 2, 0, 3)),
            "wvT": np.ascontiguousarray(
                WvT.astype(bf).reshape(NCT, 128, 256).transpose(1, 0, 2)),
            "bqk": np.ascontiguousarray(bqk.reshape(4, 128).T.astype(np.float32)),
            "bv": np.ascontiguousarray(
                np.broadcast_to(bv[None, :].astype(np.float32), (128, 256))),
            "rope": ropeT,
            "masks": masks,
            "wpT": np.ascontiguousarray(
                WpT.astype(bf).reshape(2, 128, 1024).transpose(1, 0, 2)),
        })
    return in_maps


def kernel(x, W_attn, b_attn, W_proj, b_proj):
    if "nc" not in _CACHE:
        _CACHE["nc"] = _build_nc()
    nc = _CACHE["nc"]
    in_maps = _host_inputs(x, W_attn, b_attn, W_proj, b_proj)
    res = run_bass_kernel_spmd(nc, in_maps, list(range(8)), trace=TRACE)
    _CACHE["last"] = res
    y = np.zeros((B, T, C), np.float32)
    for c in range(8):
        y[c // 4] += res.results[c]["yT"].astype(np.float32).T
    y += b_proj.astype(np.float32)
    return y


# revision 29
# speedup vs baseline: 1.1664x; 1.1664x over previous
"""Causal self-attention (B=2, T=2048, C=1024, NH=16, HD=64) on 8 TRN2 cores.

Sharding: core c -> batch b = c//4, head group j = c%4 (4 heads: 4j..4j+3).
Each core computes its batch's QKV projection for its 4 heads, rope, causal
attention in S^T layout (k on partitions, q on free dim), and a partial
output projection y_part^T = Wp_slice^T.T @ attT. Host sums the 4 per-batch
partials and adds b_proj.

Fused per-qc pipeline: for each 512-token q chunk, attention (both head
pairs), rowsum-normalize, output projection and DMA-out all overlap with the
next chunk's attention. QK/V projection blocks are interleaved as PE filler
work into the exp-bound gaps of the attention loop.

Device layouts (per core, t = 2048 tokens of its batch):
  xT   [128, 8, 2048]  bf16   x[b].T tiled over 8 c-tiles
  q/k  [128, 2, 2048]  bf16   head-pair dims on partitions, rope applied
  v    [128, 64, 65]   bf16   [tok-part, u=(tt,hp,h), 64 dims + ones col]
  S^T  psum [128, 1024] f32   [kt 128 x (h0 512q | h1 512q)]
  P^T  [128, 1024] bf16       exp(S^T/8), causal-masked on DVE
  PV   psum [65, 512] x2      rows 0-63 att^T, row 64 rowsum (ones col)
  attbf [128, 2, 512] bf16    per-qc normalized att^T, consumed by E
  yT   [1024, 2048] bf16      per-qc column blocks DMA'd as produced
"""
import numpy as np
import ml_dtypes
from contextlib import ExitStack

import concourse.bass as bass
import concourse.mybir as mybir
import concourse.tile as tile
from concourse import bacc
from concourse.bass_utils import run_bass_kernel_spmd

F32 = mybir.dt.float32
BF16 = mybir.dt.bfloat16
AF = mybir.ActivationFunctionType
ALU = mybir.AluOpType

B, T, C = 2, 2048, 1024
NH, HD = 16, 64
TL = 2048          # per-core token count (one batch)
NCT = C // 128     # 8 contraction tiles
NTC = TL // 512    # 4 q-chunks of 512
NTT = TL // 128    # 16 token tiles of 128

TRACE = False      # set by test harness for profiling runs
_CACHE = {}


def _build_nc():
    nc = bacc.Bacc("TRN2", target_bir_lowering=False, debug=False)
    xT_d = nc.dram_tensor("xT", [128, NCT, TL], BF16, kind="ExternalInput").ap()
    wqk_d = nc.dram_tensor("wqkT", [128, 4, NCT, 128], BF16, kind="ExternalInput").ap()
    wv_d = nc.dram_tensor("wvT", [128, NCT, 256], BF16, kind="ExternalInput").ap()
    bqk_d = nc.dram_tensor("bqk", [128, 4], F32, kind="ExternalInput").ap()
    bv_d = nc.dram_tensor("bv", [128, 256], F32, kind="ExternalInput").ap()
    rope_d = nc.dram_tensor("rope", [128, TL], BF16, kind="ExternalInput").ap()
    masks_d = nc.dram_tensor("masks", [128, 4, 1024], BF16, kind="ExternalInput").ap()
    wp_d = nc.dram_tensor("wpT", [128, 2, 1024], BF16, kind="ExternalInput").ap()
    yT_d = nc.dram_tensor("yT", [1024, TL], BF16, kind="ExternalOutput").ap()
    rs_dram = nc.dram_tensor("rs_scratch", [4, TL], F32)

    with tile.TileContext(nc) as tc, ExitStack() as ctx:
        sb = ctx.enter_context(tc.tile_pool(name="sb", bufs=1))
        ptp = ctx.enter_context(tc.tile_pool(name="ptp", bufs=6))
        abp = ctx.enter_context(tc.tile_pool(name="abp", bufs=2))
        atp = ctx.enter_context(tc.tile_pool(name="atp", bufs=4))
        rbp = ctx.enter_context(tc.tile_pool(name="rbp", bufs=4))
        ytp = ctx.enter_context(tc.tile_pool(name="ytp", bufs=4))

        xT = sb.tile([128, NCT, TL], BF16)
        wqk = sb.tile([128, 4, NCT, 128], BF16)
        wv = sb.tile([128, NCT, 256], BF16)
        bqk = sb.tile([128, 4], F32)
        bv = sb.tile([128, 256], F32)
        rope = sb.tile([128, TL], BF16)
        masks = sb.tile([128, 4, 1024], BF16)
        wp = sb.tile([128, 2, 1024], BF16)
        q_sb = sb.tile([128, 2, TL], BF16)
        k_sb = sb.tile([128, 2, TL], BF16)
        v_sb = sb.tile([128, 4 * NTT, 65], BF16)
        rs_sb = sb.tile([128, 512], F32)    # rows 0/32 <- rowsums of h0/h1
        rsr_sb = sb.tile([128, 512], F32)   # reciprocal of rs_sb
        ones_sb = sb.tile([128, 64], BF16)  # lhsT for PE rowsum broadcast
        rsrb_sb = sb.tile([128, 512], BF16)  # bf16 recip rows (tail bcast rhs)
        rsc_sb = sb.tile([128, 512], F32)   # PE-broadcast recip (last chunk)

        # ---- input DMA, priority-ordered ----
        # DMA_DIRECT2D occupies the issuing ring for the transfer. Wave 1
        # (everything the first blocks + D(0,hp0) start need) fans out over
        # 5 rings; tensor/vector only carry wave 1 so their compute streams
        # aren't delayed. Waves 2/3 round-robin sync/gpsimd/scalar.
        nc.vector.memset(v_sb[:, :, 64:65], 1.0)
        nc.vector.memset(rs_sb, 1.0)  # keep unused partitions finite for recip
        nc.vector.memset(ones_sb, 1.0)

        r1 = [nc.sync, nc.gpsimd, nc.scalar]
        ri = [0]

        def dma1(out, in_):
            r1[ri[0] % 3].dma_start(out=out, in_=in_)
            ri[0] += 1

        dma1(bqk, bqk_d)
        dma1(wqk[:, 2], wqk_d[:, 2])
        dma1(wqk[:, 0], wqk_d[:, 0])
        for half in range(2):
            dma1(wv[:, half * 4:(half + 1) * 4, :],
                 wv_d[:, half * 4:(half + 1) * 4, :])
        for ct in range(NCT):  # x for q-chunk 0
            dma1(xT[:, ct, 0:512], xT_d[:, ct, 0:512])
        dma1(bv, bv_d)
        dma1(rope, rope_d)
        dma1(masks[:, 0, :], masks_d[:, 0, :])

        r2 = [nc.sync, nc.gpsimd, nc.scalar]
        ri2 = [0]

        def dma(out, in_):
            r2[ri2[0] % 3].dma_start(out=out, in_=in_)
            ri2[0] += 1

        for m in (3, 1):       # head pair 1 weights
            dma(wqk[:, m], wqk_d[:, m])
        for d in range(1, 4):
            dma(masks[:, d, :], masks_d[:, d, :])
        for tc4 in (1, 2, 3):
            for ct in range(NCT):
                sl = slice(tc4 * 512, (tc4 + 1) * 512)
                dma(xT[:, ct, sl], xT_d[:, ct, sl])
            if tc4 == 1:
                dma(wp[:, 0, :], wp_d[:, 0, :])
                dma(wp[:, 1, :], wp_d[:, 1, :])

        with tc.tile_pool(name="pds", bufs=2, space="PSUM") as pds, \
             tc.tile_pool(name="pdv", bufs=1, space="PSUM") as pdv, \
             tc.tile_pool(name="fill", bufs=1, space="PSUM") as fill:

            def c_tile(tt):
                """V projection for one token tile (8 MMs + fused bias add)."""
                ps = fill.tile([128, 512], F32, tag=f"f{tt % 2}",
                               name=f"pc_{tt}")[:, 0:256]
                for ct in range(NCT):
                    nc.tensor.matmul(
                        ps, xT[:, ct, tt * 128:(tt + 1) * 128], wv[:, ct, :],
                        start=(ct == 0), stop=(ct == NCT - 1),
                        skip_group_check=True)
                nc.vector.tensor_add(
                    v_sb[:, 4 * tt:4 * tt + 4, 0:64],
                    ps.rearrange("p (a b) -> p a b", a=4),
                    bv.rearrange("p (a b) -> p a b", a=4))

            def b_block(m, tc4):
                """QK projection for one 512-col q/k chunk of m-tile m."""
                for f in b_chunks(m, tc4):
                    f()

            def b_chunks(m, tc4):
                """b_block split into 4 2-MM closures so filler pops during
                the DMA-bound phase only wait on freshly-arrived x chunks
                instead of wedging the in-order PE queue."""
                sl = slice(tc4 * 512, (tc4 + 1) * 512)
                state = {}

                def go(ci):
                    if ci == 0:
                        state["ps"] = fill.tile([128, 512], F32,
                                                tag=f"f{m % 2}",
                                                name=f"pb_{m}_{tc4}")
                    ps = state["ps"]
                    for ct in (2 * ci, 2 * ci + 1):
                        nc.tensor.matmul(
                            ps, wqk[:, m, ct, :], xT[:, ct, sl],
                            start=(ct == 0), stop=(ct == NCT - 1),
                            skip_group_check=True)
                    if ci == 3:
                        dest = q_sb if m < 2 else k_sb
                        nc.vector.scalar_tensor_tensor(
                            out=dest[:, m % 2, sl], in0=ps,
                            scalar=bqk[:, m:m + 1], in1=rope[:, sl],
                            op0=ALU.add, op1=ALU.mult)
                return [lambda ci=ci: go(ci) for ci in range(4)]

            def c_chunks(tt):
                """c_tile split into 2 4-MM closures (same reason)."""
                state = {}

                def go(ci):
                    if ci == 0:
                        state["ps"] = fill.tile(
                            [128, 512], F32, tag=f"f{tt % 2}",
                            name=f"pc_{tt}")[:, 0:256]
                    ps = state["ps"]
                    for ct in range(4 * ci, 4 * ci + 4):
                        nc.tensor.matmul(
                            ps, xT[:, ct, tt * 128:(tt + 1) * 128],
                            wv[:, ct, :],
                            start=(ct == 0), stop=(ct == NCT - 1),
                            skip_group_check=True)
                    if ci == 1:
                        nc.vector.tensor_add(
                            v_sb[:, 4 * tt:4 * tt + 4, 0:64],
                            ps.rearrange("p (a b) -> p a b", a=4),
                            bv.rearrange("p (a b) -> p a b", a=4))
                return [lambda ci=ci: go(ci) for ci in range(2)]

            def norm(qc, hp, pv):
                """Copy att^T out of psum (releases pv fast; split DVE/ACT so
                the next head pair's PV never stalls), rowsum recip + recip
                broadcast; returns (attT, rsb) for the final mul.
                Cross-partition copies keep 32-aligned partition shifts.
                Broadcast goes through a DRAM bounce (latency hidden by the
                pipeline) except for the very last chunk, where a PE
                broadcast matmul keeps the chain short and the PE warm."""
                qsl = slice(qc * 512, (qc + 1) * 512)
                att = atp.tile([128, 512], F32, tag="att", name=f"att_{qc}_{hp}")
                nc.vector.tensor_copy(att[0:64, :], pv[0][0:64, :])
                nc.vector.tensor_copy(rs_sb[0:1, :], pv[0][64:65, :])
                nc.scalar.copy(att[64:128, :], pv[1][0:64, :])
                nc.scalar.copy(rs_sb[32:33, :], pv[1][64:65, :])
                nc.vector.reciprocal_approx_fast(rsr_sb, rs_sb)
                if qc == 3 and hp == 1:
                    nc.scalar.copy(rsrb_sb[0:33, :], rsr_sb[0:33, :])
                    bc = fill.tile([128, 512], F32, tag="f0", name="bc_tail")
                    for h in range(2):
                        nc.tensor.matmul(
                            bc[h * 64:(h + 1) * 64, :],
                            ones_sb[32 * h:32 * h + 1, :],
                            rsrb_sb[32 * h:32 * h + 1, :])
                    nc.vector.tensor_copy(rsc_sb, bc)
                    return att, rsc_sb
                rsb = rbp.tile([128, 512], F32, tag="rsb", name=f"rsb_{qc}_{hp}")
                for h in range(2):
                    u4 = 2 * hp + h
                    nc.gpsimd.dma_start(
                        out=rs_dram[u4:u4 + 1, qsl], in_=rsr_sb[32 * h:32 * h + 1, :])
                    bc_ap = bass.AP(tensor=rs_dram, offset=u4 * TL + qc * 512,
                                    ap=[[0, 64], [1, 512]])
                    nc.gpsimd.dma_start(out=rsb[h * 64:(h + 1) * 64, :], in_=bc_ap)
                return att, rsb

            def attbf_mul(ab, hp, att, rsb):
                nc.vector.tensor_mul(ab[:, hp, :], att, rsb)

            def e_mt(qc, mt, ab):
                """Output projection for one 128-row m-tile of this q chunk."""
                ps = fill.tile([128, 512], F32, tag=f"f{mt % 2}",
                               name=f"pe_{qc}_{mt}")
                for hp in range(2):
                    nc.tensor.matmul(
                        ps, wp[:, hp, mt * 128:(mt + 1) * 128], ab[:, hp, :],
                        start=(hp == 0), stop=(hp == 1), skip_group_check=True)
                yt = ytp.tile([128, 512], BF16, tag="yt", name=f"yt_{qc}_{mt}")
                nc.vector.tensor_copy(yt, ps)
                nc.sync.dma_start(
                    out=yT_d[mt * 128:(mt + 1) * 128, qc * 512:(qc + 1) * 512],
                    in_=yt)

            # ---- prework: only what D(0,hp0,kt0) needs; the rest of
            # chunk 0's blocks drain as early fillers ----
            b_block(2, 0)
            b_block(0, 0)
            c_tile(0)
            # `must`: B/C work the next chunk depends on — flushed before
            # that chunk starts. `soft`: deferred normalize muls + E chunks
            # (pop later, by which time their rsb broadcast has landed).
            must = ([lambda: c_tile(1), lambda: c_tile(2)]
                    + b_chunks(3, 0)
                    + [lambda: c_tile(3)] + b_chunks(1, 0))
            soft = []

            def pop_filler(n=1):
                for _ in range(n):
                    if must:
                        must.pop(0)()
                    elif soft:
                        soft.pop(0)()

            ab = None
            for qc in range(4):
                qsl = slice(qc * 512, (qc + 1) * 512)
                n_kt = 4 * (qc + 1)
                if qc > 0:  # stragglers this chunk depends on
                    for f in must:
                        f()
                    must = []
                if qc < 3:
                    for m in (0, 2, 1, 3):
                        must += b_chunks(m, qc + 1)
                    for tt in range(4 * qc + 4, 4 * qc + 8):
                        must += c_chunks(tt)
                for hp in range(2):
                    pv = [pdv.tile([65, 512], F32, tag=f"pv{h}",
                                   name=f"pv_{qc}_{hp}_{h}")
                          for h in range(2)]
                    for kt in range(n_kt):
                        ksl = slice(kt * 128, (kt + 1) * 128)
                        # diagonal staircase: tile kt covers k in
                        # [128kt,128kt+128); q cols j < 128d of this chunk
                        # get zero contribution, so S/exp/mask/PV all shrink
                        # to the valid cols [128d:512).
                        d = kt - 4 * qc  # >= 0 on the diagonal band
                        off = 128 * d if d > 0 else 0
                        sps = pds.tile([128, 1024], F32, tag="sps",
                                       name=f"sps_{qc}_{hp}_{kt}")
                        for h in range(2):
                            hsl = slice(h * 64, (h + 1) * 64)
                            nc.tensor.matmul(
                                sps[:, h * 512 + off:(h + 1) * 512],
                                k_sb[hsl, hp, ksl],
                                q_sb[hsl, hp, qc * 512 + off:(qc + 1) * 512])
                        pt = ptp.tile([128, 1024], BF16, tag="pt",
                                      name=f"pt_{qc}_{hp}_{kt}")
                        if d >= 2:  # split exp pays once >=256 cols are dead
                            for h in range(2):
                                hs = slice(h * 512 + off, (h + 1) * 512)
                                nc.scalar.activation(
                                    pt[:, hs], sps[:, hs], AF.Exp,
                                    bias=0.0, scale=0.125)
                        else:
                            nc.scalar.activation(pt, sps, AF.Exp,
                                                 bias=0.0, scale=0.125)
                        if d == 0:
                            nc.vector.tensor_mul(pt, pt, masks[:, 0, :])
                        elif d > 0:
                            for h in range(2):
                                hs = slice(h * 512 + off, (h + 1) * 512)
                                nc.vector.tensor_mul(
                                    pt[:, hs], pt[:, hs], masks[:, d, hs])
                        for h in range(2):
                            u = (kt * 2 + hp) * 2 + h
                            nc.tensor.matmul(
                                pv[h][:, off:512],
                                v_sb[:, u, :],
                                pt[:, h * 512 + off:(h + 1) * 512],
                                start=(kt == 0), stop=(kt == n_kt - 1),
                                skip_group_check=True)
                        pop_filler(3 if qc == 0 else 2)
                    att, rsb = norm(qc, hp, pv)
                    if hp == 0:
                        ab = abp.tile([128, 2, 512], BF16, tag="ab",
                                      name=f"ab_{qc}")
                    # deferred: pops after next chunk's B/C blocks, by which
                    # time the rsb broadcast DMA has landed (no DVE stall)
                    soft.append(
                        lambda ab=ab, hp=hp, att=att, rsb=rsb:
                        attbf_mul(ab, hp, att, rsb))
                # projection of this q chunk interleaves into the next chunk
                soft += [(lambda qc=qc, mt=mt, ab=ab: e_mt(qc, mt, ab))
                         for mt in range(8)]
            for f in must + soft:
                f()
    nc.compile()
    return nc


def _rope_T():
    theta = 1.0 / (10000.0 ** (2.0 * np.arange(0, HD // 2, dtype=np.float32) / HD))
    seq = np.arange(1, T + 1, dtype=np.float32)
    ang = np.einsum('n,d->nd', seq, theta)
    ang = np.concatenate([ang, ang], axis=-1)
    f = (np.cos(ang) + np.sin(ang)).astype(np.float32)  # [T, 64]
    return np.concatenate([f.T, f.T], axis=0)           # [128, T]


def _host_inputs(x, W_attn, b_attn, W_proj, b_proj):
    bf = ml_dtypes.bfloat16
    ropeT = _rope_T().astype(bf)
    masks = np.empty((128, 4, 1024), dtype=bf)
    kp = np.arange(128)[:, None]
    qf = np.arange(512)[None, :]
    for d in range(4):
        m = ((kp + 128 * d) <= qf).astype(np.float32)
        masks[:, d, :] = np.concatenate([m, m], axis=1).astype(bf)

    in_maps = []
    for c in range(8):
        b, j = divmod(c, 4)
        hs = [4 * j + i for i in range(4)]
        xT = np.ascontiguousarray(x[b].T).astype(bf)          # [1024, TL]
        q_rows = np.concatenate([W_attn[64 * h:64 * (h + 1)] for h in hs], 0)
        k_rows = np.concatenate([W_attn[C + 64 * h:C + 64 * (h + 1)] for h in hs], 0)
        WqkT = np.concatenate([q_rows, k_rows], 0).T          # [1024, 512]
        bqk = np.concatenate(
            [np.concatenate([b_attn[64 * h:64 * (h + 1)] for h in hs]),
             np.concatenate([b_attn[C + 64 * h:C + 64 * (h + 1)] for h in hs])])
        v_rows = np.concatenate([W_attn[2 * C + 64 * h:2 * C + 64 * (h + 1)] for h in hs], 0)
        WvT = v_rows.T                                        # [1024, 256]
        bv = np.concatenate([b_attn[2 * C + 64 * h:2 * C + 64 * (h + 1)] for h in hs])
        WpT = np.concatenate([W_proj[:, 64 * h:64 * (h + 1)] for h in hs], 1).T  # [256,1024]
        in_maps.append({
            "xT": np.ascontiguousarray(
                xT.reshape(NCT, 128, TL).transpose(1, 0, 2)),
            # [128, 4, NCT, 128]: wqkT[p, m, ct, j] = WqkT[ct*128+p, m*128+j]
            "wqkT": np.ascontiguousarray(
                WqkT.astype(bf).reshape(NCT, 128, 4, 128).transpose(1, 2, 0, 3)),
            "wvT": np.ascontiguousarray(
                WvT.astype(bf).reshape(NCT, 128, 256).transpose(1, 0, 2)),
            "bqk": np.ascontiguousarray(bqk.reshape(4, 128).T.astype(np.float32)),
            "bv": np.ascontiguousarray(
                np.broadcast_to(bv[None, :].astype(np.float32), (128, 256))),
            "rope": ropeT,
            "masks": masks,
            "wpT": np.ascontiguousarray(
                WpT.astype(bf).reshape(2, 128, 1024).transpose(1, 0, 2)),
        })
    return in_maps


def kernel(x, W_attn, b_attn, W_proj, b_proj):
    if "nc" not in _CACHE:
        _CACHE["nc"] = _build_nc()
    nc = _CACHE["nc"]
    in_maps = _host_inputs(x, W_attn, b_attn, W_proj, b_proj)
    res = run_bass_kernel_spmd(nc, in_maps, list(range(8)), trace=TRACE)
    _CACHE["last"] = res
    y = np.zeros((B, T, C), np.float32)
    for c in range(8):
        y[c // 4] += res.results[c]["yT"].astype(np.float32).T
    y += b_proj.astype(np.float32)
    return y
